# revision 29
# baseline (speedup 1.0000x reference)
"""Demodulated 3x3 convolution Trainium2 kernel (v2).

Data-parallel over batch: 16 samples -> 8 cores x 2 samples.

Design notes (cost-model driven):
  * All DMAs move 512B contiguous runs and are batched 16 rows per
    dma_start: avoids the <512B 2x DMA penalty and amortizes the ~630ns
    shared-HWDGE hold per dma_start. Input blocks land in raw (row,
    parity, c) layout at full rate; one DVE copy per block relayouts to
    parity-major bf16 so the PE transpose inputs are contiguous (BIR
    requires the stationary matmul operand to merge to one free dim).
  * Conv runs as matmuls with out = [q-pixel partitions, (row, parity,
    f) cols]: channel-major bf16 slabs are the stationary operand,
    per-sample modulated+demodulated weights stream as rhs. Output
    lands pixel-major -> no output transpose at all.
  * Slabs hold both rows of a pair interleaved by pixel (col 1+2q+par),
    so the 3 pixel-taps per parity are stride-2 windows. Cross-row taps
    (ky=0/2 reaching outside the row pair) read the neighbor pair's
    slab directly with zero-padded weight halves - no extra copies.
  * Demodulation is folded into the weights at prologue; the epilogue
    is a single DVE add of broadcast bias from PSUM into the output
    staging tile, which DMAs out 16 rows at a time.

Per-pair steady state: PE 2 bf16 transposes + 18 matmuls ~= 1792 cyc
(~747ns @2.4GHz); ACT 1 slab copy; DVE 1 epilogue add (+1 relayout per
8 pairs); 2 dma_starts per 8 pairs. TimelineSim: ~214us/core (PE 89%
busy, DMA 88% -- ridge), vs ~710us for the previous version.
"""

import math
import os
import sys

os.environ["BASS_NEVER_TRACE"] = "1"

for _p in ("/opt/trn_rl_repo",):
    if _p not in sys.path:
        sys.path.insert(0, _p)

import numpy as np

import concourse.bass as bass
import concourse.bacc as bacc
import concourse.mybir as mybir
import concourse.tile as tile
from concourse.bass_utils import run_bass_kernel_spmd

B, H, W, CIN = 16, 256, 256, 64
LATENT, F, KK = 512, 64, 3
NCORES = 8
BS = B // NCORES  # samples per core

NPAIR = H // 2          # 128 row pairs per sample
NPTOT = BS * NPAIR      # 256 pairs per core
RBLK = 16               # rows per DMA block
PBLK = RBLK // 2        # 8 pairs per block
NQ = W // 2             # 128 pixel pairs per row
SLAB_BUFS = 5           # slab ring depth (halo zeroing is per slot)

F32 = mybir.dt.float32
F32R = mybir.dt.float32r
BF16 = mybir.dt.bfloat16
AF = mybir.ActivationFunctionType
ALU = mybir.AluOpType

_CACHE = {}


def _r(ap):
    return ap.bitcast(F32R)


def _build_nc():
    nc = bacc.Bacc("TRN2", target_bir_lowering=False, debug=False)

    x_h = nc.dram_tensor("x", [BS, H, W, CIN], F32, kind="ExternalInput")
    lat_h = nc.dram_tensor("lat", [BS, LATENT], F32, kind="ExternalInput")
    dw_h = nc.dram_tensor("dw", [LATENT, CIN], F32, kind="ExternalInput")
    db_h = nc.dram_tensor("db", [CIN], F32, kind="ExternalInput")
    ck_h = nc.dram_tensor("ck", [KK, KK, CIN, F], F32, kind="ExternalInput")
    bi_h = nc.dram_tensor("bi", [F], F32, kind="ExternalInput")
    id_h = nc.dram_tensor("ident", [128, 128], F32, kind="ExternalInput")
    out_h = nc.dram_tensor("out", [BS, H, W, F], F32, kind="ExternalOutput")

    const_k = math.sqrt(2.0) / math.sqrt(KK * KK * CIN)
    inv_sqrt_lat = 1.0 / math.sqrt(LATENT)

    with tile.TileContext(nc) as tc:
        with (
            tc.tile_pool(name="const", bufs=1) as cpool,
            tc.tile_pool(name="stage", bufs=2) as stpool,
            tc.tile_pool(name="slab", bufs=SLAB_BUFS) as slpool,
            tc.tile_pool(name="onat", bufs=2) as onpool,
        ):
            stg_live = {}   # block -> stg tile
            raw_live = {}   # block -> raw staging tile
            NBLK = NPTOT // PBLK

            def fetch_dma(blk):
                if blk >= NBLK or blk in raw_live:
                    return
                raw = stpool.tile(
                    [128, RBLK, 2, CIN], F32, tag="raw", name="raw"
                )
                sb, rb = divmod(blk, H // RBLK)
                r0 = rb * RBLK
                nc.sync.dma_start(
                    raw[:],
                    x_h[sb, r0 : r0 + RBLK, :, :].rearrange(
                        "r (q t) c -> q r t c", t=2
                    ),
                )
                raw_live[blk] = raw
                raw_live.pop(blk - 2, None)

            def fetch_relayout(blk):
                if blk >= NBLK or blk in stg_live:
                    return
                fetch_dma(blk)
                stg = stpool.tile(
                    [128, 2, RBLK, CIN], BF16, tag="stg", name="stg"
                )
                nc.vector.tensor_copy(
                    stg[:].rearrange("q t r c -> q r t c"), raw_live[blk][:]
                )
                stg_live[blk] = stg
                stg_live.pop(blk - 2, None)

            ident = cpool.tile([128, 128], F32)
            identb = cpool.tile([128, 128], BF16)

            # per-sample demodulated+modulated weights (bf16)
            wmdS = []   # [128(K=(rpar,c)), 2(r), 3(kx), 64(f)]
            wxuT = []   # [128, 3, 64] upper-cross (ky=0 at upper K)
            wxlT = []   # [128, 3, 64] lower-cross (ky=2 at lower K)
            biasB4 = cpool.tile([128, 4, 64], F32)  # bias bcast (r, t, f)

            with (
                tc.tile_pool(name="pro", bufs=1) as pro,
                tc.tile_pool(name="prop", bufs=1, space="PSUM") as prop,
            ):
                # ---- critical-path DMAs (style chain first)
                latD = pro.tile([128, 4, BS], F32)
                for b in range(BS):
                    nc.sync.dma_start(
                        latD[:, :, b],
                        lat_h[b, :].rearrange("(j p) -> p j", p=128),
                    )
                dwD = pro.tile([128, 4, CIN], F32)
                nc.sync.dma_start(
                    dwD[:], dw_h[:].rearrange("(j p) f -> p j f", p=128)
                )
                db_t = pro.tile([CIN, 1], F32)
                nc.sync.dma_start(
                    db_t[:], db_h[:].rearrange("(c u) -> c u", u=1)
                )
                ckT = []
                for ky in range(KK):
                    ck1t = pro.tile([64, 3, 64], F32, tag=f"ck{ky}",
                                    name=f"ck{ky}")
                    nc.sync.dma_start(
                        ck1t[:],
                        ck_h[ky, :, :, :].rearrange("kx c f -> c kx f"),
                    )
                    ckT.append(ck1t)
                bias1 = pro.tile([1, 64], F32)
                nc.sync.dma_start(
                    bias1[:], bi_h[:].rearrange("(u f) -> u f", u=1)
                )
                # input block 0 + identity after the critical-path DMAs
                fetch_dma(0)
                nc.sync.dma_start(ident[:], id_h[:])
                nc.vector.tensor_copy(identb[:], ident[:])

                # ---- style = (lat @ dw) * inv_sqrt_lat*const_k + db*const_k
                ps_style = prop.tile([CIN, BS], F32)
                for j in range(4):
                    nc.tensor.matmul(
                        ps_style[:],
                        dwD[:, j, :],
                        latD[:, j, :],
                        start=(j == 0),
                        stop=(j == 3),
                    )
                db_s = pro.tile([CIN, 1], F32)
                nc.vector.tensor_scalar_mul(db_s[:], db_t[:], const_k)
                styleC2 = pro.tile([128, BS], F32)
                nc.vector.tensor_scalar(
                    styleC2[0:64, :],
                    ps_style[:],
                    inv_sqrt_lat * const_k,
                    db_s[:],
                    op0=ALU.mult,
                    op1=ALU.add,
                )
                nc.vector.tensor_copy(styleC2[64:128, :], styleC2[0:64, :])

                # ---- base (unmodulated) weight layouts, f32 (DVE shuffles)
                # K lower 64 = even in-row channels, upper = odd in-row.
                # r=0 (even out-row): ky=1 @ lower, ky=2 @ upper
                # r=1 (odd out-row):  ky=0 @ lower, ky=1 @ upper
                kmainS = pro.tile([128, 2, 3, 64], F32)
                nc.vector.tensor_copy(kmainS[0:64, 0, :, :], ckT[1][:])
                nc.vector.tensor_copy(kmainS[0:64, 1, :, :], ckT[0][:])
                nc.vector.tensor_copy(kmainS[64:128, 0, :, :], ckT[2][:])
                nc.vector.tensor_copy(kmainS[64:128, 1, :, :], ckT[1][:])
                kxuS = pro.tile([128, 3, 64], F32)
                nc.vector.memset(kxuS[0:64, :, :], 0.0)
                nc.vector.tensor_copy(kxuS[64:128, :, :], ckT[0][:])
                kxlS = pro.tile([128, 3, 64], F32)
                nc.vector.memset(kxlS[64:128, :, :], 0.0)
                nc.vector.tensor_copy(kxlS[0:64, :, :], ckT[2][:])

                # ---- constants + bias broadcast (K=1 outer products)
                ones1 = pro.tile([1, 128], F32)
                nc.vector.memset(ones1[:], 1.0)
                epsP = pro.tile([1, 1], F32)
                nc.vector.memset(epsP[:], 1e-8)
                onesb = pro.tile([128, 1], BF16)
                nc.vector.memset(onesb[:], 1.0)
                psB = prop.tile([128, 4, 64], F32)
                for u in range(4):
                    nc.tensor.matmul(
                        psB[:, u, :],
                        ones1[:],
                        bias1[:],
                        start=(u == 0),
                        stop=(u == 3),
                        skip_group_check=True,
                    )
                nc.vector.tensor_copy(biasB4[:], psB[:])

                # relayout block 0 once prologue DVE chain is queued
                fetch_relayout(0)

                # ---- per-sample: modulate, demod, fold demod into weights
                for s in range(BS):
                    wmF = pro.tile([128, 2, 3, 64], F32, tag=f"wmF{s}")
                    nc.vector.tensor_scalar_mul(
                        wmF[:], kmainS[:], styleC2[:, s : s + 1]
                    )
                    wxuF = pro.tile([128, 3, 64], F32, tag=f"wxuF{s}")
                    nc.vector.tensor_scalar_mul(
                        wxuF[:], kxuS[:], styleC2[:, s : s + 1]
                    )
                    wxlF = pro.tile([128, 3, 64], F32, tag=f"wxlF{s}")
                    nc.vector.tensor_scalar_mul(
                        wxlF[:], kxlS[:], styleC2[:, s : s + 1]
                    )

                    # demod_f = rsqrt(sum_{ky,kx,c} wmod^2 + 1e-8)
                    # r=0 cols cover ky{1,2}; r=1 lower-K covers ky0.
                    sqM = pro.tile([128, 2, 3, 64], BF16, tag=f"sqM{s}")
                    nc.vector.tensor_mul(sqM[:], wmF[:], wmF[:])
                    psd = prop.tile([1, 64], F32, tag=f"psd{s}")
                    for dxi in range(3):
                        nc.tensor.matmul(
                            psd[:],
                            onesb[:],
                            sqM[:, 0, dxi, :],
                            start=(dxi == 0),
                            stop=False,
                        )
                    for dxi in range(3):
                        nc.tensor.matmul(
                            psd[:],
                            onesb[0:64, :],
                            sqM[0:64, 1, dxi, :],
                            start=False,
                            stop=(dxi == 2),
                        )
                    rt = pro.tile([1, 64], F32, tag=f"rt{s}")
                    nc.scalar.activation(
                        rt[:], psd[:], AF.Sqrt, bias=epsP[:]
                    )
                    dmrow = pro.tile([1, 64], F32, tag=f"dmrow{s}")
                    nc.vector.reciprocal(dmrow[:], rt[:])
                    psDm = prop.tile([128, 64], F32, tag=f"psDm{s}")
                    nc.tensor.matmul(
                        psDm[:], ones1[:], dmrow[:], start=True, stop=True
                    )
                    dmB = pro.tile([128, 64], F32, tag=f"dmB{s}")
                    nc.vector.tensor_copy(dmB[:], psDm[:])

                    dmB6 = (
                        dmB[:]
                        .rearrange("p (u v f) -> p u v f", u=1, v=1)
                        .broadcast_to((128, 2, 3, 64))
                    )
                    dmB3 = (
                        dmB[:]
                        .rearrange("p (u f) -> p u f", u=1)
                        .broadcast_to((128, 3, 64))
                    )
                    wmd = cpool.tile([128, 2, 3, 64], BF16, tag=f"wmd{s}")
                    nc.vector.tensor_mul(wmd[:], wmF[:], dmB6)
                    wxu = cpool.tile([128, 3, 64], BF16, tag=f"wxu{s}")
                    wxl = cpool.tile([128, 3, 64], BF16, tag=f"wxl{s}")
                    nc.vector.tensor_mul(wxu[:], wxuF[:], dmB3)
                    nc.vector.tensor_mul(wxl[:], wxlF[:], dmB3)
                    wmdS.append(wmd)
                    wxuT.append(wxu)
                    wxlT.append(wxl)

            biasV = biasB4[:].rearrange("p (r t) f -> p r t f", t=2)

            # ---- main loop ----
            with (
                tc.tile_pool(name="tpsum", bufs=3, space="PSUM") as tpsum,
                tc.tile_pool(name="cpsum", bufs=3, space="PSUM") as cpsum,
            ):
                T_live = {}     # pair j -> slab tile
                cv_live = {}    # pair g -> psum tile
                onat_cur = [None]
                slab_count = 0

                def win(Tt, p2, dx):
                    # window AP for source pixel (2q + p2 + dx), q=0..127
                    o = p2 + dx + 1
                    j0, par = divmod(o, 2)
                    return Tt[:, j0 : j0 + 128, par]

                for t in range(-2, NPTOT + 1):
                    j = t + 2
                    # ---- input stage: DMA block / transpose / slab copy
                    if 0 <= j <= NPTOT - 1:
                        blk, jl = divmod(j, PBLK)
                        if jl == 0:
                            # prefetch next block; current one was fetched
                            # a block ago (full-rate DMA into raw layout +
                            # one DVE relayout to parity-major bf16)
                            fetch_relayout(blk + 1)
                        stg = stg_live[blk]

                        psT = tpsum.tile([128, 2, 128], BF16)
                        for p2 in range(2):
                            nc.tensor.transpose(
                                psT[:, p2, :],
                                stg[:, p2, 2 * jl : 2 * jl + 2, :],
                                identb[:],
                            )
                        Tt = slpool.tile([128, 129, 2], BF16, tag="T")
                        if slab_count < SLAB_BUFS:
                            # zero halo pixels (-1 and 256) once per slot
                            nc.vector.memset(Tt[:, 0, 0:1], 0.0)
                            nc.vector.memset(Tt[:, 128, 1:2], 0.0)
                        slab_count += 1
                        # data cols flat 1..256: pixel w at flat 1+w
                        dst = (
                            Tt[:]
                            .rearrange("p j t -> p (j t)")[:, 1:257]
                            .rearrange("p (q t) -> p t q", t=2)
                        )
                        nc.scalar.activation(dst, psT[:, :, :], AF.Copy)
                        T_live[j] = Tt
                        T_live.pop(j - 4, None)

                    # ---- matmul stage for pair g = t
                    if 0 <= t <= NPTOT - 1:
                        s, p = divmod(t, NPAIR)
                        cv = cpsum.tile([128, 2, 2, 64], F32, tag="cv")
                        Tc = T_live[t]
                        Tp = T_live.get(t - 1) if p >= 1 else None
                        Tn = T_live.get(t + 1) if p <= NPAIR - 2 else None
                        wmd, wxu, wxl = wmdS[s], wxuT[s], wxlT[s]
                        for p2 in range(2):
                            if Tp is not None and Tn is not None:
                                # interior: fat mains (both r), thin crosses
                                for dxi in range(3):
                                    nc.tensor.matmul(
                                        cv[:, :, p2, :],
                                        win(Tc, p2, dxi - 1),
                                        wmd[:, :, dxi, :],
                                        start=(dxi == 0),
                                        stop=False,
                                        skip_group_check=True,
                                    )
                                for dxi in range(3):
                                    nc.tensor.matmul(
                                        cv[:, 0, p2, :],
                                        win(Tp, p2, dxi - 1),
                                        wxu[:, dxi, :],
                                        start=False,
                                        stop=(dxi == 2),
                                        skip_group_check=True,
                                    )
                                for dxi in range(3):
                                    nc.tensor.matmul(
                                        cv[:, 1, p2, :],
                                        win(Tn, p2, dxi - 1),
                                        wxl[:, dxi, :],
                                        start=False,
                                        stop=(dxi == 2),
                                        skip_group_check=True,
                                    )
                            else:
                                # boundary pair: per-r mains so each
                                # quadrant gets clean start/stop flags
                                for rr in range(2):
                                    Tx = Tp if rr == 0 else Tn
                                    wx = wxu if rr == 0 else wxl
                                    for dxi in range(3):
                                        nc.tensor.matmul(
                                            cv[:, rr, p2, :],
                                            win(Tc, p2, dxi - 1),
                                            wmd[:, rr, dxi, :],
                                            start=(dxi == 0),
                                            stop=(dxi == 2 and Tx is None),
                                            skip_group_check=True,
                                        )
                                    if Tx is not None:
                                        for dxi in range(3):
                                            nc.tensor.matmul(
                                                cv[:, rr, p2, :],
                                                win(Tx, p2, dxi - 1),
                                                wx[:, dxi, :],
                                                start=False,
                                                stop=(dxi == 2),
                                                skip_group_check=True,
                                            )
                        cv_live[t] = cv
                        cv_live.pop(t - 2, None)

                    # ---- epilogue + output DMA for pair e = t - 1
                    e = t - 1
                    if 0 <= e <= NPTOT - 1:
                        blkE, el = divmod(e, PBLK)
                        if el == 0:
                            onat_cur[0] = onpool.tile(
                                [128, RBLK, 2, F], F32, tag="onat", name="onat"
                            )
                        onat = onat_cur[0]
                        nc.vector.tensor_add(
                            onat[:, 2 * el : 2 * el + 2, :, :],
                            cv_live[e][:],
                            biasV,
                        )
                        if el == PBLK - 1:
                            sb, rb = divmod(blkE, H // RBLK)
                            r0 = rb * RBLK
                            nc.sync.dma_start(
                                out_h[sb, r0 : r0 + RBLK, :, :].rearrange(
                                    "r (q t) f -> q r t f", t=2
                                ),
                                onat[:],
                            )

    nc.compile()
    return nc


def _get_nc():
    if "nc" not in _CACHE:
        _CACHE["nc"] = _build_nc()
    return _CACHE["nc"]


def kernel(feature_map, latent, dense_w, dense_b, conv_kernel, bias):
    nc = _get_nc()
    feature_map = np.ascontiguousarray(feature_map, dtype=np.float32)
    latent = np.ascontiguousarray(latent, dtype=np.float32)
    ident = np.eye(128, dtype=np.float32)
    in_maps = []
    for i in range(NCORES):
        in_maps.append(
            {
                "x": np.ascontiguousarray(feature_map[BS * i : BS * (i + 1)]),
                "lat": np.ascontiguousarray(latent[BS * i : BS * (i + 1)]),
                "dw": np.ascontiguousarray(dense_w, dtype=np.float32),
                "db": np.ascontiguousarray(dense_b, dtype=np.float32),
                "ck": np.ascontiguousarray(conv_kernel, dtype=np.float32),
                "bi": np.ascontiguousarray(bias, dtype=np.float32),
                "ident": ident,
            }
        )
    res = run_bass_kernel_spmd(nc, in_maps, core_ids=list(range(NCORES)))
    outs = [res.results[i]["out"] for i in range(NCORES)]
    full = np.concatenate(outs, axis=0)
    if getattr(res, "exec_time_ns", None):
        kernel.last_exec_time_ns = res.exec_time_ns
    return full


kernel.last_exec_time_ns = None
